# revision 15
# baseline (speedup 1.0000x reference)
"""Trainium2 Bass kernel for ColumnConsistencyLoss (segment_reduce).

Problem: B=16, T=8192, C=128.
  probs = softmax(logits, -1)           # (N, C), N = B*T = 131072
  per column-id c (segment): n_c = #valid tokens, S_c = sum w*p, Q_c = sum w*p^2
  col_var_c = (sum_j Q_cj - sum_j S_cj^2 / n_safe_c) / (n_safe_c * C)
  loss = mean over columns with n_c > 1 of col_var_c

Sharding ("compacted"): only VALID tokens (w=1, ~50% of N) contribute to
the loss, so the host gathers the valid tokens and distributes them
evenly over the 8 cores (any token->core mapping is legal: the segment
sums are permutation invariant).  Each core gets a fixed capacity of
J=65 tiles x 128 tokens = 8320 tokens (valid count is ~8192 +- 23 per
core); unused slots are padded with zero logits and a zero one-hot
column so they contribute nothing.  In the (astronomically unlikely)
case that the valid count exceeds the total capacity, the overflow
tokens are folded in exactly on the host.

Device kernel per core (tokens on partitions, row-major [P, j, C] tiles
so the matmul moving operand is contiguous; whole-core SBUF buffers so
there is no pool-rotation serialization):
  - DMA L [P, J, C] bf16 in 4 pieces (sync ring), M [P, J, C] fp8
    one-hot in 1 piece (gpsimd ring, off the compute engines)
  - ACT:  E = exp(L) per chunk        (one big-free instruction each)
  - DVE:  d = rowsum(E) via bf16 halving tree h1,h2,h3 (2x mode) +
          f32 reduce over the remaining 16
  - DVE:  rv = 1/d (reciprocal_approx_fast); ACT: rb2 = bf16(rv) x2
          (the normalizer is stored as an adjacent PAIR so the
          broadcast multiply below can use the DVE 2x mode: all
          operands are 2-byte with a packed stride-1 last dim)
  - DVE:  rhs[.., 0, :] = E * pairbcast(rb2)    (normalized probs p)
  - ACT or DVE (per-chunk knob): rhs[.., 1, :] = p^2
  - PE:   per tile jj: psum[c, 0:2C] += M[:,jj,:]^T @ rhs[:, jj, :, :]
          accumulating S rows (F 0:C) and Q rows (F C:2C) over all tiles
Host finalizes: exact n via bincount, ssd = rowsum(Q) - rowsum(S^2)/n,
masked mean over columns with n>1.
"""

import numpy as np
import ml_dtypes

NCORES = 8
P = 128           # partitions (tokens per tile)
C = 128           # columns / segments
H = C // 2
B, T = 16, 8192
N_TOK = B * T
CHUNKS = (2, 6, 12, 15, 15, 15)       # token tiles per compute chunk
ACT_SQ = (False, True, False, True, True, False)  # square on ACT?
NPSUM = 4                              # parallel PSUM accumulators
M_SPLIT = 24                           # one-hot DMA piece boundary (tiles)
L_FP8 = True                           # stream logits as fp8e4m3 (rel err ~6e-3)
J = sum(CHUNKS)                        # 65 tiles
TOK_CAP = J * P                        # 8320 tokens per core
F2 = 2 * C

TRACE = False          # set True (e.g. from test.py) to capture NTFF profile
TRACE_TMPDIR = None    # where trace/NEFF artifacts land when TRACE is set
LAST_RESULT = None     # BassKernelResults of the last run (for profiling)

_NC_CACHE = {}


def build_nc(chunks=CHUNKS, act_sq=ACT_SQ, m_split=M_SPLIT, l_fp8=L_FP8):
    """Build + compile the Bass program (SPMD; same NEFF on all cores)."""
    from concourse import bacc, mybir
    import concourse.tile as tile

    f32 = mybir.dt.float32
    bf16 = mybir.dt.bfloat16
    fp8 = mybir.dt.float8e4
    ldt = fp8 if l_fp8 else bf16
    Exp = mybir.ActivationFunctionType.Exp
    Square = mybir.ActivationFunctionType.Square
    Alu = mybir.AluOpType

    j_full = sum(chunks)
    nchunk = len(chunks)
    offs = [sum(chunks[:k]) for k in range(nchunk)]

    nc = bacc.Bacc("TRN2", target_bir_lowering=False, debug=False,
                   enable_asserts=False)

    lg_d = nc.dram_tensor("logits", [P * j_full * C], ldt,
                          kind="ExternalInput")
    m_d = nc.dram_tensor("m8", [P * j_full * C], fp8, kind="ExternalInput")
    sq_d = nc.dram_tensor("sq_out", [C, F2], f32, kind="ExternalOutput")

    lg_ap = lg_d[:].rearrange("(p j c) -> p j c", j=j_full, c=C)
    m_ap = m_d[:].rearrange("(p j c) -> p j c", j=j_full, c=C)

    with tile.TileContext(nc) as tc:
        with (
            tc.tile_pool(name="buf", bufs=1) as bufp,
            tc.tile_pool(name="psum", bufs=1, space="PSUM") as psump,
        ):
            # Stripe the accumulation over NPSUM full banks: consecutive
            # matmuls hit different banks, dodging the read-modify-write
            # turnaround of back-to-back accumulation into one bank.
            # [C, 512] f32 is exactly one 2KB/partition bank.
            psums = [psump.tile([C, 512], f32, tag=f"ps{i}", name=f"ps{i}")
                     for i in range(NPSUM)]

            L = bufp.tile([P, j_full, C], ldt)
            M8 = bufp.tile([P, j_full, C], fp8)
            E = bufp.tile([P, j_full, C], bf16)
            RHS = bufp.tile([P, j_full, 2, C], bf16)
            h1 = bufp.tile([P, j_full, H], bf16)
            h2 = bufp.tile([P, j_full, 32], bf16)
            h3 = bufp.tile([P, j_full, 16], bf16)
            dd = bufp.tile([P, j_full], f32)
            rv = bufp.tile([P, j_full], f32)
            rb2 = bufp.tile([P, j_full, 2], bf16)
            junk = bufp.tile([P, 2], f32)

            # Warm-ups on garbage data while the DMA is in flight: pull
            # the ACT exp-table load and the DVE custom-op library load
            # off the critical path.
            nc.scalar.activation(junk[:], junk[:], Exp)
            nc.vector.reciprocal_approx_fast(junk[:], junk[:])

            # All input DMA rides ONE HWDGE ring (sync) so arrival order
            # is exactly program order: logits pieces (chunk-aligned,
            # small first so compute starts early) with the one-hot
            # pieces interleaved just-in-time for the matmuls.
            for k in range(nchunk):
                a, b = offs[k], offs[k] + chunks[k]
                nc.sync.dma_start(L[:, a:b, :], lg_ap[:, a:b, :])
                if k == 2:
                    nc.sync.dma_start(M8[:, 0:m_split, :],
                                      m_ap[:, 0:m_split, :])
                if k == 4:
                    nc.sync.dma_start(M8[:, m_split:j_full, :],
                                      m_ap[:, m_split:j_full, :])

            def pair(ap):  # [P, cj, C] -> [P, cj, 64, 2] (packed pairs)
                return ap.rearrange("p j (h t) -> p j h t", t=2)

            nc.scalar.activation(E[:, 0:chunks[0], :], L[:, 0:chunks[0], :],
                                 Exp)
            for k, cj in enumerate(chunks):
                a, b = offs[k], offs[k] + cj
                # d = rowsum(E): bf16 halving tree (2x mode) + packed tail
                nc.vector.tensor_tensor(h1[:, a:b, :], E[:, a:b, 0:H],
                                        E[:, a:b, H:C], op=Alu.add)
                nc.vector.tensor_tensor(h2[:, a:b, :], h1[:, a:b, 0:32],
                                        h1[:, a:b, 32:64], op=Alu.add)
                nc.vector.tensor_tensor(h3[:, a:b, :], h2[:, a:b, 0:16],
                                        h2[:, a:b, 16:32], op=Alu.add)
                nc.vector.tensor_reduce(dd[:, a:b], h3[:, a:b, :],
                                        axis=mybir.AxisListType.X, op=Alu.add)
                nc.vector.reciprocal_approx_fast(rv[:, a:b], dd[:, a:b])
                nc.gpsimd.tensor_copy(
                    rb2[:, a:b, :],
                    rv[:, a:b, None].to_broadcast([P, cj, 2]))

                nc.vector.tensor_tensor(
                    pair(RHS[:, a:b, 0, :]), pair(E[:, a:b, :]),
                    rb2[:, a:b, None, :].to_broadcast([P, cj, H, 2]),
                    op=Alu.mult)
                # next chunk's exp goes ahead of this chunk's square in the
                # in-order ACT queue so exp is never stalled behind DVE
                if k + 1 < nchunk:
                    a2, b2 = offs[k + 1], offs[k + 1] + chunks[k + 1]
                    nc.scalar.activation(E[:, a2:b2, :], L[:, a2:b2, :], Exp)
                if act_sq[k]:
                    nc.scalar.activation(RHS[:, a:b, 1, :], RHS[:, a:b, 0, :],
                                         Square)
                else:
                    nc.vector.tensor_tensor(
                        RHS[:, a:b, 1, :], RHS[:, a:b, 0, :],
                        RHS[:, a:b, 0, :], op=Alu.mult)

                for jj in range(a, b):
                    nc.tensor.matmul(
                        psums[jj % NPSUM][:, 0:F2], M8[:, jj, :],
                        RHS[:, jj, :, :],
                        start=(jj < NPSUM), stop=(jj >= j_full - NPSUM))

            # DVE may read only ONE PSUM operand per instruction
            t1 = bufp.tile([C, F2], f32)
            t2 = bufp.tile([C, F2], f32)
            out_t = bufp.tile([C, F2], f32)
            nc.vector.tensor_copy(t1[:], psums[0][:, 0:F2])
            nc.vector.tensor_tensor(t2[:], t1[:], psums[1][:, 0:F2],
                                    op=Alu.add)
            nc.vector.tensor_tensor(t1[:], t2[:], psums[2][:, 0:F2],
                                    op=Alu.add)
            nc.vector.tensor_tensor(out_t[:], t1[:], psums[3][:, 0:F2],
                                    op=Alu.add)
            nc.sync.dma_start(sq_d[:], out_t[:])

    nc.compile()
    return nc


def _get_nc():
    key = (CHUNKS, ACT_SQ, M_SPLIT, L_FP8)
    if key not in _NC_CACHE:
        _NC_CACHE[key] = build_nc(CHUNKS, ACT_SQ, M_SPLIT, L_FP8)
    return _NC_CACHE[key]


def _pack_core(Lv, Sv):
    """Pack one core's [J, P, C] logits + [J, P] segments into the
    (p, j, c) DMA layout."""
    fp8 = ml_dtypes.float8_e4m3
    lpk = np.ascontiguousarray(Lv.transpose(1, 0, 2)).ravel()
    M = np.zeros((J, P, C), dtype=fp8)
    valid = Sv >= 0
    jj, pp = np.nonzero(valid)
    M[jj, pp, Sv[jj, pp]] = fp8(1.0)
    mpk = np.ascontiguousarray(M.transpose(1, 0, 2)).ravel()
    return lpk, mpk


def kernel(column_logits, column_assignments, valid_mask):
    global LAST_RESULT
    from concourse.bass_utils import run_bass_kernel_spmd

    ldt = ml_dtypes.float8_e4m3 if L_FP8 else ml_dtypes.bfloat16

    logits = np.asarray(column_logits, dtype=np.float32).reshape(N_TOK, C)
    seg = np.asarray(column_assignments).reshape(N_TOK).astype(np.int64)
    w = np.asarray(valid_mask).reshape(N_TOK).astype(bool)

    vidx = np.nonzero(w)[0]
    cap = NCORES * TOK_CAP
    dev_idx = vidx[:cap]
    ov_idx = vidx[cap:]          # overflow (essentially never non-empty)

    nv = dev_idx.size
    # Compacted per-core arrays [J, P, C] / [J, P]; seg = -1 marks padding.
    Lv = np.zeros((cap, C), dtype=ldt)
    Lv[:nv] = logits[dev_idx].astype(ldt)
    Sv = np.full(cap, -1, dtype=np.int64)
    Sv[:nv] = seg[dev_idx]

    in_maps = []
    for i in range(NCORES):
        sl = slice(i * TOK_CAP, (i + 1) * TOK_CAP)
        lpk, mpk = _pack_core(Lv[sl].reshape(J, P, C),
                              Sv[sl].reshape(J, P))
        in_maps.append({"logits": lpk, "m8": mpk})

    nc = _get_nc()
    res = run_bass_kernel_spmd(nc, in_maps, list(range(NCORES)), trace=TRACE,
                               tmpdir=TRACE_TMPDIR)
    LAST_RESULT = res

    SQ = np.zeros((C, F2), np.float64)
    for rm in res.results:
        SQ += np.asarray(rm["sq_out"], dtype=np.float64)
    S = SQ[:, 0:C].copy()
    Q = SQ[:, C:F2].copy()

    if ov_idx.size:              # exact host fold-in of overflow tokens
        Lo = logits[ov_idx].astype(np.float64)
        Eo = np.exp(Lo)
        po = Eo / Eo.sum(axis=1, keepdims=True)
        so = seg[ov_idx]
        np.add.at(S, so, po)
        np.add.at(Q, so, po * po)

    n = np.bincount(seg[w], minlength=C).astype(np.float64)
    n_safe = np.maximum(n, 1.0)
    ssd_sum = Q.sum(axis=1) - (S * S).sum(axis=1) / n_safe
    col_var = ssd_sum / (n_safe * C)
    has_multi = n > 1.0
    count = has_multi.sum()
    total = np.where(has_multi, col_var, 0.0).sum()
    loss = total / max(count, 1.0) if count > 0 else 0.0
    return np.asarray(loss, dtype=np.float32)
